# revision 15
# baseline (speedup 1.0000x reference)
"""Trainium2 Bass kernel for BaseGraphPromptEncoder (ragged scatter to padded slab).

Op: given x[N, D] and sorted batch_index[N] with G graphs, produce
  node_embeds[G, M, D]  (graph-g rows packed at the front, zeros after)
  node_mask[G, M] int32 (1 for valid rows)

Because batch_index is sorted, each graph's rows are a contiguous block of x,
so the scatter is G contiguous block copies + zero fills of the padding.
Sharding: split the feature dim D=512 across 8 cores (64 cols each) so every
core runs the SAME program (per-graph counts are baked in as compile-time
constants at trace time; the program is rebuilt per distinct batch_index).

Per core (all DMA, raw Bass, two HWDGE issue queues):
  - 64 contiguous DRAM->DRAM copies (x rows -> slab rows), interleaved with
  - 64 zero-fill DMAs from an SBUF zero tile (padding rows), plus
  - node_mask computed on GPSIMD (iota < counts) and stored with one DMA.
"""

import os
import sys

sys.path.insert(0, "/opt/trn_rl_repo")

import numpy as np
import concourse.bass as bass
import concourse.mybir as mybir
from concourse.bass_utils import run_bass_kernel_spmd

N_CORES = 8
N = 200000
D = 512
G = 64
M = 4096
DC = D // N_CORES  # 64 columns per core

_NC_CACHE: dict[bytes, object] = {}
LAST_RESULT = None  # BassKernelResults of the most recent run (for test harness)


def _build_nc(counts: np.ndarray):
    # The spmd runner pre-zeros + donates ExternalOutput buffers, so the gap
    # zero-fill DMAs could be skipped (BGPE_SKIP_ZERO_FILL=1 experiment knob);
    # default is to write every output byte on device.
    skip_zero = os.environ.get("BGPE_SKIP_ZERO_FILL") == "1"
    starts = np.zeros(G, dtype=np.int64)
    starts[1:] = np.cumsum(counts)[:-1]

    nc = bass.Bass()
    xc = nc.dram_tensor("xc", [N, DC], mybir.dt.float32, kind="ExternalInput")
    cnt = nc.dram_tensor("cnt", [G, 1], mybir.dt.float32, kind="ExternalInput")
    emb = nc.dram_tensor("emb", [G, M, DC], mybir.dt.float32, kind="ExternalOutput")
    msk = nc.dram_tensor("msk", [G, M], mybir.dt.int32, kind="ExternalOutput")

    data_jobs = []  # (g, count, start)
    zero_jobs = []  # (g, count, pad_rows)
    for g in range(G):
        c, s = int(counts[g]), int(starts[g])
        if c > 0:
            data_jobs.append((g, c, s))
        if c < M and not skip_zero:
            zero_jobs.append((g, c, M - c))

    # cnt load tracked by cnt_sem only (its completion is implied by the
    # m_sem -> mask-store chain); everything else incs dma_sem.
    n_zero_dmas = 0
    for _, c, r in zero_jobs:
        rows15 = r - (r % 15)
        if (rows15 // 15) * DC > 4 * M:
            n_zero_dmas += 1
        else:
            n_zero_dmas += (1 if rows15 else 0) + (1 if r % 15 else 0)
    n_dmas = len(data_jobs) + n_zero_dmas + 1  # + mask store
    tot = 16 * n_dmas

    # A DMA's descriptors are handed to SDMA engines round-robin starting at
    # engine 0, one per SBUF partition chunk. Engine 15 streams ~15% slower
    # than the rest, and the D2D data copies (16 descriptors) already pin its
    # share - so issue each zero fill as a 15-descriptor DMA (engines 0-14
    # only) plus a tiny remainder DMA.
    def issue_zero(eng, g, c, r):
        # row-aligned 15-descriptor split: descriptors stay 256B-aligned and
        # the remainder (r%15 rows) is a single small row-aligned DMA.
        rows15 = r - (r % 15)
        f = (rows15 // 15) * DC
        if f > 4 * M:  # gap too large for the 15-wide window: use 16-wide
            eng.dma_start(emb[g, c:M, :], z[0:16, 0 : 4 * r]).then_inc(
                dma_sem, 16
            )
            return
        if rows15:
            eng.dma_start(
                emb[g, c : c + rows15, :], z[0:15, 0:f]
            ).then_inc(dma_sem, 16)
        if r % 15:
            eng.dma_start(
                emb[g, c + rows15 : M, :], z[15:16, 0 : (r % 15) * DC]
            ).then_inc(dma_sem, 16)

    def issue_data(eng, g, c, s):
        eng.dma_start(emb[g, 0:c, :], xc[s : s + c, :]).then_inc(dma_sem, 16)

    def queue_jobs(eng, jobs, first_n=6):
        """Issue `first_n` data jobs, wait for the zero tile, then interleave."""
        d = [j for j in jobs if j[0] == "d"]
        zz = [j for j in jobs if j[0] == "z"]
        for _, g, c, s in d[:first_n]:
            issue_data(eng, g, c, s)
        d = d[first_n:]
        if zz:
            eng.wait_ge(z_sem, 1)
        k = 0
        while d or zz:
            if k % 2 == 0 and d:
                _, g, c, s = d.pop(0)
                issue_data(eng, g, c, s)
            elif zz:
                _, g, c, r = zz.pop(0)
                issue_zero(eng, g, c, r)
            elif d:
                _, g, c, s = d.pop(0)
                issue_data(eng, g, c, s)
            k += 1

    sync_jobs = []
    scalar_jobs = []
    for i, (g, c, s) in enumerate(data_jobs):
        (sync_jobs if i % 2 == 0 else scalar_jobs).append(("d", g, c, s))
    for i, (g, c, r) in enumerate(zero_jobs):
        (sync_jobs if i % 2 == 1 else scalar_jobs).append(("z", g, c, r))

    with (
        nc.semaphore("dma_sem") as dma_sem,
        nc.semaphore("cnt_sem") as cnt_sem,
        nc.semaphore("z_sem") as z_sem,
        nc.semaphore("m_sem") as m_sem,
        nc.sbuf_tensor("z", [128, 4 * M], mybir.dt.float32) as z,
        nc.sbuf_tensor("ctt", [G, 1], mybir.dt.float32) as ctt,
        nc.sbuf_tensor("it", [G, M], mybir.dt.float32) as it,
        nc.sbuf_tensor("mt", [G, M], mybir.dt.int32) as mt,
        nc.Block() as block,
    ):

        @block.vector
        def _(vector):
            vector.memset(z[:], 0.0).then_inc(z_sem, 1)

        @block.gpsimd
        def _(gpsimd):
            gpsimd.iota(
                it[:],
                [[1, M]],
                channel_multiplier=0,
                allow_small_or_imprecise_dtypes=True,
            )
            gpsimd.wait_ge(cnt_sem, 16)
            gpsimd.tensor_scalar(
                mt[:], it[:], ctt[:], None, mybir.AluOpType.is_lt
            ).then_inc(m_sem, 1)

        @block.sync
        def _(sync):
            sync.dma_start(ctt[:], cnt[:]).then_inc(cnt_sem, 16)
            queue_jobs(sync, sync_jobs)
            sync.wait_ge(dma_sem, tot)

        @block.scalar
        def _(scalar):
            queue_jobs(scalar, scalar_jobs)
            scalar.wait_ge(m_sem, 1)
            scalar.dma_start(msk[:], mt[:]).then_inc(dma_sem, 16)
            scalar.wait_ge(dma_sem, tot)

    return nc


def kernel(x, batch_index, num_graphs, max_num_nodes):
    global LAST_RESULT
    x = np.ascontiguousarray(np.asarray(x, dtype=np.float32))
    bi = np.asarray(batch_index).astype(np.int64)
    g_, m_ = int(np.asarray(num_graphs)), int(np.asarray(max_num_nodes))
    assert g_ == G and m_ == M, f"hardcoded for G={G}, M={M}, got {g_}, {m_}"
    assert x.shape == (N, D), f"hardcoded for x{(N, D)}, got {x.shape}"
    assert bi.shape == (N,)

    counts = np.bincount(bi, minlength=G).astype(np.int64)
    assert counts.max() <= M, "graph larger than max_num_nodes"
    assert np.all(np.diff(bi) >= 0), "batch_index must be sorted"

    key = counts.tobytes() + os.environ.get("BGPE_SKIP_ZERO_FILL", "").encode()
    nc = _NC_CACHE.get(key)
    if nc is None:
        nc = _build_nc(counts)
        _NC_CACHE[key] = nc

    cnt_f32 = counts.astype(np.float32).reshape(G, 1)
    in_maps = [
        {
            "xc": np.ascontiguousarray(x[:, c * DC : (c + 1) * DC]),
            "cnt": cnt_f32,
        }
        for c in range(N_CORES)
    ]
    res = run_bass_kernel_spmd(nc, in_maps, core_ids=list(range(N_CORES)))
    LAST_RESULT = res

    out = np.empty((G, M, D), dtype=np.float32)
    for c in range(N_CORES):
        out[:, :, c * DC : (c + 1) * DC] = res.results[c]["emb"]
    mask = np.ascontiguousarray(res.results[0]["msk"].astype(np.int32))
    return out, mask


# revision 17
# speedup vs baseline: 1.4726x; 1.4726x over previous
"""Trainium2 Bass kernel for BaseGraphPromptEncoder (ragged scatter to padded slab).

Op: given x[N, D] and sorted batch_index[N] with G graphs, produce
  node_embeds[G, M, D]  (graph-g rows packed at the front, zeros after)
  node_mask[G, M] int32 (1 for valid rows)

Because batch_index is sorted, each graph's rows are a contiguous block of x,
so the scatter is G contiguous block copies + zero fills of the padding.
Sharding: split the feature dim D=512 across 8 cores (64 cols each) so every
core runs the SAME program (per-graph counts are baked in as compile-time
constants at trace time; the program is rebuilt per distinct batch_index).

Per core (all DMA, raw Bass, two HWDGE issue queues):
  - 64 contiguous DRAM->DRAM copies (x rows -> slab rows), interleaved with
  - 64 zero-fill DMAs from an SBUF zero tile (padding rows), plus
  - node_mask computed on GPSIMD (iota < counts) and stored with one DMA.
"""

import os
import sys

sys.path.insert(0, "/opt/trn_rl_repo")

import numpy as np
import concourse.bass as bass
import concourse.mybir as mybir
from concourse.bass_utils import run_bass_kernel_spmd

N_CORES = 8
N = 200000
D = 512
G = 64
M = 4096
DC = D // N_CORES  # 64 columns per core

_NC_CACHE: dict[bytes, object] = {}
LAST_RESULT = None  # BassKernelResults of the most recent run (for test harness)


def _build_nc(counts: np.ndarray):
    # The spmd runner pre-zeros + donates ExternalOutput buffers, so the gap
    # zero-fill DMAs could be skipped (BGPE_SKIP_ZERO_FILL=1 experiment knob);
    # default is to write every output byte on device.
    skip_zero = os.environ.get("BGPE_SKIP_ZERO_FILL") == "1"
    starts = np.zeros(G, dtype=np.int64)
    starts[1:] = np.cumsum(counts)[:-1]

    nc = bass.Bass()
    xc = nc.dram_tensor("xc", [N, DC], mybir.dt.float32, kind="ExternalInput")
    cnt = nc.dram_tensor("cnt", [G, 1], mybir.dt.float32, kind="ExternalInput")
    emb = nc.dram_tensor("emb", [G, M, DC], mybir.dt.float32, kind="ExternalOutput")
    msk = nc.dram_tensor("msk", [G, M], mybir.dt.int32, kind="ExternalOutput")

    data_jobs = []  # (g, count, start)
    zero_jobs = []  # (g, count, pad_rows)
    for g in range(G):
        c, s = int(counts[g]), int(starts[g])
        if c > 0:
            data_jobs.append((g, c, s))
        if c < M and not skip_zero:
            zero_jobs.append((g, c, M - c))

    # cnt load tracked by cnt_sem only (its completion is implied by the
    # m_sem -> mask-store chain); everything else incs dma_sem.
    n_dmas = len(data_jobs) + len(zero_jobs) + 1  # + mask store
    tot = 16 * n_dmas

    # A DMA's descriptors are handed to SDMA engines round-robin starting at
    # engine 0, one per SBUF partition chunk. Engine 15 streams ~15% slower
    # than the rest, and the D2D data copies (16 descriptors) already pin its
    # share - so issue each zero fill as a 15-descriptor DMA (engines 0-14
    # only) plus a tiny remainder DMA.
    def issue_zero(eng, g, c, r):
        # gap = r rows x DC cols = r*DC elems from a 16-partition window of
        # the zero tile: 16 descriptors of 16r bytes each (one per SDMA
        # engine), fat enough to stream at line rate.
        eng.dma_start(emb[g, c:M, :], z[0:16, 0 : 4 * r]).then_inc(dma_sem, 16)

    def issue_data(eng, g, c, s):
        eng.dma_start(emb[g, 0:c, :], xc[s : s + c, :]).then_inc(dma_sem, 16)

    def queue_jobs(eng, jobs, first_n=6):
        """Issue `first_n` data jobs, wait for the zero tile, then interleave."""
        d = [j for j in jobs if j[0] == "d"]
        zz = [j for j in jobs if j[0] == "z"]
        for _, g, c, s in d[:first_n]:
            issue_data(eng, g, c, s)
        d = d[first_n:]
        if zz:
            eng.wait_ge(z_sem, 1)
        k = 0
        while d or zz:
            if k % 2 == 0 and d:
                _, g, c, s = d.pop(0)
                issue_data(eng, g, c, s)
            elif zz:
                _, g, c, r = zz.pop(0)
                issue_zero(eng, g, c, r)
            elif d:
                _, g, c, s = d.pop(0)
                issue_data(eng, g, c, s)
            k += 1

    sync_jobs = []
    scalar_jobs = []
    for i, (g, c, s) in enumerate(data_jobs):
        (sync_jobs if i % 2 == 0 else scalar_jobs).append(("d", g, c, s))
    for i, (g, c, r) in enumerate(zero_jobs):
        (sync_jobs if i % 2 == 1 else scalar_jobs).append(("z", g, c, r))

    with (
        nc.semaphore("dma_sem") as dma_sem,
        nc.semaphore("cnt_sem") as cnt_sem,
        nc.semaphore("z_sem") as z_sem,
        nc.semaphore("m_sem") as m_sem,
        nc.sbuf_tensor("z", [128, 4 * M], mybir.dt.float32) as z,
        nc.sbuf_tensor("ctt", [G, 1], mybir.dt.float32) as ctt,
        nc.sbuf_tensor("it", [G, M], mybir.dt.float32) as it,
        nc.sbuf_tensor("mt", [G, M], mybir.dt.int32) as mt,
        nc.Block() as block,
    ):

        @block.vector
        def _(vector):
            vector.memset(z[:], 0.0).then_inc(z_sem, 1)

        @block.gpsimd
        def _(gpsimd):
            gpsimd.iota(
                it[:],
                [[1, M]],
                channel_multiplier=0,
                allow_small_or_imprecise_dtypes=True,
            )
            gpsimd.wait_ge(cnt_sem, 16)
            gpsimd.tensor_scalar(
                mt[:], it[:], ctt[:], None, mybir.AluOpType.is_lt
            ).then_inc(m_sem, 1)

        @block.sync
        def _(sync):
            sync.dma_start(ctt[:], cnt[:]).then_inc(cnt_sem, 16)
            queue_jobs(sync, sync_jobs)
            sync.wait_ge(dma_sem, tot)

        @block.scalar
        def _(scalar):
            queue_jobs(scalar, scalar_jobs)
            scalar.wait_ge(m_sem, 1)
            scalar.dma_start(msk[:], mt[:]).then_inc(dma_sem, 16)
            scalar.wait_ge(dma_sem, tot)

    return nc


def kernel(x, batch_index, num_graphs, max_num_nodes):
    global LAST_RESULT
    x = np.ascontiguousarray(np.asarray(x, dtype=np.float32))
    bi = np.asarray(batch_index).astype(np.int64)
    g_, m_ = int(np.asarray(num_graphs)), int(np.asarray(max_num_nodes))
    assert g_ == G and m_ == M, f"hardcoded for G={G}, M={M}, got {g_}, {m_}"
    assert x.shape == (N, D), f"hardcoded for x{(N, D)}, got {x.shape}"
    assert bi.shape == (N,)

    counts = np.bincount(bi, minlength=G).astype(np.int64)
    assert counts.max() <= M, "graph larger than max_num_nodes"
    assert np.all(np.diff(bi) >= 0), "batch_index must be sorted"

    key = counts.tobytes() + os.environ.get("BGPE_SKIP_ZERO_FILL", "").encode()
    nc = _NC_CACHE.get(key)
    if nc is None:
        nc = _build_nc(counts)
        _NC_CACHE[key] = nc

    cnt_f32 = counts.astype(np.float32).reshape(G, 1)
    in_maps = [
        {
            "xc": np.ascontiguousarray(x[:, c * DC : (c + 1) * DC]),
            "cnt": cnt_f32,
        }
        for c in range(N_CORES)
    ]
    res = run_bass_kernel_spmd(nc, in_maps, core_ids=list(range(N_CORES)))
    LAST_RESULT = res

    out = np.empty((G, M, D), dtype=np.float32)
    for c in range(N_CORES):
        out[:, :, c * DC : (c + 1) * DC] = res.results[c]["emb"]
    mask = np.ascontiguousarray(res.results[0]["msk"].astype(np.int32))
    return out, mask
